# revision 40
# baseline (speedup 1.0000x reference)
"""BatchAll triplet loss (multi-module variant) on 8 Trainium2 NeuronCores.

Math: labels = [0..191, 0..191] -- every label appears exactly twice, so each
anchor i has exactly ONE valid positive j = (i+192) % 384.  The (i,j,k) cubic
triplet tensor collapses to an (i,k) problem:

    loss_terms[i,k] = relu(d(i, p(i)) - d(i,k) + margin) * pm[i,k] * valid[i,k]
    out = sum(loss_terms) / (count(loss_terms > EPS) + EPS)

where valid excludes k in {i, p(i)} and pm = tile(weight, (2,2)).

With G the raw Gram matrix and rn = 1/||e||:
    d(i,k) = sqrt(max(2 + t1[i,k], 0)),   t1[i,k] = -2 * rn_i * rn_k * G[i,k]

Weighting trick: with pmn = -pm,  relu(dpos+m-d)*pm == max((d-(dpos+m))*pmn, 0)
and count(lw > EPS) == count((d-(dpos+m))*pmn > EPS).

Sharding: anchors i are blocked over the 8 cores (48 each).  Each core gets the
full embedding set ROTATED so its anchor slab is local indices 0..47 and the
positives are at 192..239; shipped in bf16 in BOTH layouts (row-major for the
norms, D-major for the Gram), pre-tiled into [128, x] SBUF images so one DMA
covers several 128-row chunks.  Each core emits per-anchor [sum, count] partials
([48,2]); the host reduces all 8*48 rows and forms sum/(count+EPS).

bf16 notes: inputs are ~N(0,1); the Gram/norms are bf16 x bf16 -> f32 PSUM
accumulation, distances carry ~0.1% relative noise into a 2e-2 tolerance.

Perf structure (from NTFF traces):
- bf16 halves DMA bytes and runs the PE at full rate (fp32 matmul is 1/4 rate;
  fp8 is numerically unusable here: relu thresholds amplify embedding noise).
- dual-layout embeddings ship as [128, x] DRAM images so one dma_start covers
  several 128-row chunks; R chunks lead both HWDGE rings (the norm path is the
  long pole), E chunks follow, pm last.
- norms: three DVE square-reduces (stt with accum) in DMA-arrival order; the
  ACT engine is NOT used for norms -- its table-load DMAs queue behind the
  input-DMA configs on the ACT HWDGE ring and block it until ~10.5us.
- rn_k broadcast without any transpose: rn is broadcast down 48 columns per
  chunk on DVE (ones * per-partition scalar), then one selector matmul against
  the identity per chunk gives RB_j[a,c] = rn_{128j+c} in its own PSUM tile
  (PSUM dependencies are tile-granular, so per-chunk tiles keep t1 from
  over-waiting).
- gs is a plain ACT copy fired right at Gram-end (decoupled from the rn
  chain); the -2rn_a row scale rides the per-chunk t1 scalar_tensor_tensor.
- the tail runs in pipelined halves (R half first -- the positive diagonal
  lives there): d2 on DVE, sqrt on ACT, lwpre on DVE, then relu+accum on ACT
  in parallel with the is_gt count on DVE; per-anchor [sumR,sumL,cntR,cntL]
  partials go to the host, which finishes the scalar division.
- a dummy sqrt at kernel start pulls the 1.3us ACT table load into the DMA
  shadow; dummy matmuls keep the PE HAM-unthrottled before the Gram; all
  activation biases are explicit APs (float biases pull framework const
  memsets into the measured window).
- the measured window unavoidably includes ~9us of NRT per-dispatch overhead
  (postamble semaphore-reset storm + final DMA completion).
"""

import os
import sys

for _p in ("/opt/trn_rl_repo", "/root/.axon_site/_ro/trn_rl_repo"):
    if _p not in sys.path:
        sys.path.append(_p)

# The SPMD dispatch path (bass2jax.run_bass_via_pjrt) takes jax.devices(), so
# the axon platform must stay visible.  If jax has not been initialized yet and
# JAX_PLATFORMS would hide it (e.g. "cpu"), clear the restriction.
if "jax" not in sys.modules and os.environ.get("JAX_PLATFORMS") in ("cpu",):
    del os.environ["JAX_PLATFORMS"]

import ml_dtypes
import numpy as np

import concourse.bass as bass
import concourse.tile as tile
from concourse import mybir
from concourse.bacc import Bacc
from concourse.bass_utils import run_bass_kernel_spmd

F32 = mybir.dt.float32
BF16 = mybir.dt.bfloat16
ALU = mybir.AluOpType
ACT = mybir.ActivationFunctionType

B = 192          # batch (distinct labels)
N = 2 * B        # embeddings
D = 512          # embedding dim
NCORES = 8
S = N // NCORES  # anchors per core (48)
MARGIN = 0.1
EPS = 1e-8
N_WARMUP = 10    # dummy matmuls to keep the PE out of HAM throttle
RT = N // 128    # row-layout chunks (3)
KC = D // 128    # contraction chunks (4)


def build_nc() -> bass.Bass:
    nc = Bacc()

    embr = nc.dram_tensor("embr", [128, RT * D], BF16, kind="ExternalInput")
    embt = nc.dram_tensor("embt", [128, KC * N], BF16, kind="ExternalInput")
    pmwn = nc.dram_tensor("pmwn", [S, N], BF16, kind="ExternalInput")
    out = nc.dram_tensor("out", [S, 4], F32, kind="ExternalOutput")

    with tile.TileContext(nc) as tc:
        with (
            tc.tile_pool(name="sb", bufs=1) as sb,
            tc.tile_pool(name="ps", bufs=1, space="PSUM") as ps,
        ):
            Rb = sb.tile([128, RT * D], BF16, tag="Rb")
            Eb = sb.tile([128, KC * N], BF16, tag="Eb")
            pm = sb.tile([S, N], BF16, tag="pm")

            # ---- loads: R chunks first (norm path is the long pole), split
            #      across both HWDGE rings; pm via SWDGE (needed late) ----
            nc.scalar.dma_start(out=Rb[:, D:2 * D], in_=embr[:, D:2 * D])
            nc.sync.dma_start(out=Rb[:, 0:D], in_=embr[:, 0:D])
            nc.sync.dma_start(out=Rb[:, 2 * D:3 * D], in_=embr[:, 2 * D:3 * D])
            nc.scalar.dma_start(out=Eb[:, 2 * N:4 * N], in_=embt[:, 2 * N:4 * N])
            nc.sync.dma_start(out=Eb[:, 0:2 * N], in_=embt[:, 0:2 * N])
            nc.gpsimd.dma_start(out=pm, in_=pmwn[:, :])

            # ---- identity via iotas on gpsimd + is_equal on DVE ----
            icol = sb.tile([128, 128], F32, tag="icol")
            nc.gpsimd.iota(icol, [[1, 128]], channel_multiplier=0,
                           allow_small_or_imprecise_dtypes=True)
            iprt = sb.tile([128, 1], F32, tag="iprt")
            nc.gpsimd.iota(iprt, [[0, 1]], channel_multiplier=1,
                           allow_small_or_imprecise_dtypes=True)
            ident = sb.tile([128, 128], BF16, tag="ident")
            nc.vector.tensor_scalar(ident, icol, iprt, None, op0=ALU.is_equal)

            # ---- consts / warm-up scaffolding ----
            wt = sb.tile([128, D], BF16, tag="wt")
            nc.vector.memset(wt, 1.0)
            ones_row = sb.tile([1, S], BF16, tag="ones_row")
            nc.vector.memset(ones_row, 1.0)
            twos_col = sb.tile([S, 1], F32, tag="twos_col")
            nc.vector.memset(twos_col, 2.0)
            zcol = sb.tile([128, 1], F32, tag="zcol")
            nc.vector.memset(zcol, 0.0)
            tdum = sb.tile([1, 1], F32, tag="tdum")
            # pull the ACT sqrt table early (explicit zero bias: float biases
            # pull in framework const-AP memsets, which start the measured
            # exec window earlier than necessary)
            nc.scalar.activation(tdum, twos_col[0:1, 0:1], ACT.Sqrt,
                                 bias=zcol[0:1, :], scale=1.0)

            wps = ps.tile([128, D], F32, tag="wps")
            for _ in range(N_WARMUP):
                nc.tensor.matmul(wps, wt[:, 0:128], wt, start=True, stop=True)

            # ---- norms: ns_col[p,t] = ||emb[128t+p]||^2, all on DVE in
            #      DMA-arrival order (ACT is blocked by table loads early) ----
            ns_col = sb.tile([128, RT], F32, tag="ns_col")
            junk = sb.tile([128, D], BF16, tag="junk")
            junk2 = sb.tile([128, D], BF16, tag="junk2")
            nc.vector.scalar_tensor_tensor(
                junk, Rb[:, 0:D], 1.0, Rb[:, 0:D],
                op0=ALU.mult, op1=ALU.mult, accum_out=ns_col[:, 0:1])
            nc.vector.scalar_tensor_tensor(
                junk2, Rb[:, D:2 * D], 1.0, Rb[:, D:2 * D],
                op0=ALU.mult, op1=ALU.mult, accum_out=ns_col[:, 1:2])
            nc.vector.scalar_tensor_tensor(
                junk, Rb[:, 2 * D:3 * D], 1.0, Rb[:, 2 * D:3 * D],
                op0=ALU.mult, op1=ALU.mult, accum_out=ns_col[:, 2:3])
            nrm = sb.tile([128, RT], F32, tag="nrm")
            nc.scalar.activation(nrm, ns_col, ACT.Sqrt, bias=zcol, scale=1.0)
            rn_col = sb.tile([128, RT], F32, tag="rn_col")   # 1/||e||
            nc.vector.reciprocal(rn_col, nrm)
            rnam2 = sb.tile([S, 1], F32, tag="rnam2")        # -2 * rn[slab]

            # ---- Gram slab: G[a,k] = sum_d embt[d,a] * embt[d,k];
            #      chunks 2,3 first (their DMA is first on the scalar ring) ----
            g_ps = ps.tile([S, N], F32, tag="G")
            for i, c in enumerate((2, 3, 0, 1)):
                nc.tensor.matmul(g_ps, Eb[:, c * N:c * N + S],
                                 Eb[:, c * N:(c + 1) * N],
                                 start=(i == 0), stop=(i == KC - 1))

            # ---- RB_j[a,c] = rn_{128j+c}: broadcast rn chunk j across 48
            #      columns on DVE (ones * rn as per-partition scalar), then
            #      one selector matmul against the identity per chunk.  This
            #      replaces the transpose->copy->rank-1 ping-pong. ----
            bc = sb.tile([128, RT * S], BF16, tag="bc")
            rb_ps = [ps.tile([S, 128], F32, name=f"RB{j}", tag=f"RB{j}")
                     for j in range(RT)]
            for j in (1, 2, 0):
                nc.vector.tensor_scalar(
                    bc[:, j * S:(j + 1) * S], wt[:, 0:S],
                    rn_col[:, j:j + 1], None, op0=ALU.mult)
                nc.tensor.matmul(rb_ps[j], bc[:, j * S:(j + 1) * S], ident,
                                 start=True, stop=True)
                if j == 1:
                    nc.vector.tensor_scalar_mul(rnam2, rn_col[0:S, 0:1], -2.0)

            # ---- G * -2rn_a to SBUF on ACT (Copy with per-partition scale;
            #      also leaves t1 with a single PSUM operand), in halves so
            #      t1R only waits the R half ----
            gs = sb.tile([S, N], BF16, tag="gs")
            nc.scalar.copy(gs, g_ps)

            # ---- pipelined tail in k-halves (R=[B:N] first: the positive
            #      diagonal is in R, so dpos resolves early).
            #      t1 = gs * RB;  d2 = relu(t1+2);  d = sqrt(d2)
            #      lwpre = (d - dpos_m) * pmn;  sum relu on ACT, count on DVE ----
            t1 = sb.tile([S, N], BF16, tag="T1")
            d2 = sb.tile([S, N], BF16, tag="D2")
            dms = sb.tile([S, N], BF16, tag="dms")
            lwpre = sb.tile([S, N], BF16, tag="lwpre")
            lw = sb.tile([S, N], BF16, tag="LW")
            c01 = sb.tile([S, N], BF16, tag="C01")
            dpb = sb.tile([S, S], BF16, tag="dpb")
            t1pos = sb.tile([S, 1], F32, tag="t1pos")
            dpos = sb.tile([S, 1], F32, tag="dpos")
            dpos_m = sb.tile([S, 1], F32, tag="dpos_m")
            stacked = sb.tile([S, 4], F32, tag="stacked")
            H = ((B, N), (0, B))                  # R half, then L half

            # t1 chunk-wise (each rb_ps[j] is its own PSUM tile); chunk 1
            # first -- the positive diagonal (cols 192..239) lives there
            nc.vector.scalar_tensor_tensor(
                t1[:, 128:256], rb_ps[1], rnam2, gs[:, 128:256],
                op0=ALU.mult, op1=ALU.mult)
            nc.vector.scalar_tensor_tensor(
                dpb, t1[:, B:B + S], 1.0, ident[0:S, 0:S], op0=ALU.mult,
                op1=ALU.mult, accum_out=t1pos)
            nc.vector.scalar_tensor_tensor(
                t1[:, 256:384], rb_ps[2], rnam2, gs[:, 256:384],
                op0=ALU.mult, op1=ALU.mult)
            nc.vector.tensor_scalar(d2[:, B:N], t1[:, B:N], 2.0, 0.0,
                                    op0=ALU.add, op1=ALU.max)
            nc.scalar.activation(dpos, t1pos, ACT.Sqrt, bias=twos_col, scale=1.0)
            nc.vector.scalar_tensor_tensor(
                t1[:, 0:128], rb_ps[0], rnam2, gs[:, 0:128],
                op0=ALU.mult, op1=ALU.mult)
            nc.vector.tensor_scalar_add(dpos_m, dpos, MARGIN)
            nc.vector.tensor_scalar(d2[:, 0:B], t1[:, 0:B], 2.0, 0.0,
                                    op0=ALU.add, op1=ALU.max)
            for lo, hi in H:
                nc.scalar.activation(dms[:, lo:hi], d2[:, lo:hi], ACT.Sqrt,
                                     bias=zcol[0:S, :], scale=1.0)
            for i, (lo, hi) in enumerate(H):
                nc.vector.scalar_tensor_tensor(
                    lwpre[:, lo:hi], dms[:, lo:hi], dpos_m, pm[:, lo:hi],
                    op0=ALU.subtract, op1=ALU.mult)
                nc.scalar.activation(lw[:, lo:hi], lwpre[:, lo:hi], ACT.Relu,
                                     bias=zcol[0:S, :],
                                     accum_out=stacked[:, i:i + 1])
            for i, (lo, hi) in enumerate(H):
                nc.vector.tensor_scalar(
                    c01[:, lo:hi], lwpre[:, lo:hi], EPS, 0.0, op0=ALU.is_gt,
                    op1=ALU.add, accum_out=stacked[:, 2 + i:3 + i])

            nc.sync.dma_start(out=out[:, :], in_=stacked)

    nc.finalize()
    return nc


_NC_CACHE: dict = {}


def _get_nc() -> bass.Bass:
    if "nc" not in _NC_CACHE:
        _NC_CACHE["nc"] = build_nc()
    return _NC_CACHE["nc"]


def make_in_maps(output1, output2, weight):
    o1 = np.asarray(output1, dtype=np.float32)
    o2 = np.asarray(output2, dtype=np.float32)
    w = np.asarray(weight, dtype=np.float32)

    emb = np.concatenate([o1, o2], axis=0)  # (384, 512) unnormalized
    aS = np.arange(S)

    in_maps = []
    for c in range(NCORES):
        rot = (np.arange(N) + c * S) % N                  # local -> global
        er = emb[rot].astype(ml_dtypes.bfloat16)          # (384, 512)
        # row-layout image: [128, 3*512], chunk t at cols [512t, 512t+512)
        embr_h = np.ascontiguousarray(
            er.reshape(RT, 128, D).transpose(1, 0, 2).reshape(128, RT * D))
        # D-major image: [128, 4*384], chunk c at cols [384c, 384c+384)
        et = np.ascontiguousarray(er.T)                   # (512, 384)
        embt_h = np.ascontiguousarray(
            et.reshape(KC, 128, N).transpose(1, 0, 2).reshape(128, KC * N))
        pmw = w[rot[:S] % B][:, rot % B].astype(np.float32)  # (48, 384)
        pmw[aS, aS] = 0.0          # k == i
        pmw[aS, B + aS] = 0.0      # k == p(i)
        in_maps.append({
            "embr": embr_h,
            "embt": embt_h,
            "pmwn": (-pmw).astype(ml_dtypes.bfloat16),
        })
    return in_maps


def reduce_outputs(results):
    parts = np.stack([r["out"] for r in results])         # (8, 48, 4)
    total = parts.sum(axis=(0, 1), dtype=np.float32)      # [sumR,sumL,cntR,cntL]
    s = np.float32(total[0]) + np.float32(total[1])
    c = np.float32(total[2]) + np.float32(total[3])
    return np.asarray(s / (c + np.float32(EPS)), dtype=np.float32)


def kernel(output1, output2, weight):
    in_maps = make_in_maps(output1, output2, weight)
    res = run_bass_kernel_spmd(_get_nc(), in_maps, core_ids=list(range(NCORES)))
    return reduce_outputs(res.results)
